# revision 23
# baseline (speedup 1.0000x reference)
"""Trainium2 Bass kernel for nn_AgentPredictionHead.

Model: per-agent CfC (closed-form continuous-time) prediction head:
  - initial (h, c) from agent_features via Linear(D, 2H)
  - T steps of a mixed-memory LSTM cell + CfC cell (control input constant)
  - per-step readout Linear(H, 2)

Sharding: data-parallel over agents across 8 NeuronCores (weights replicated).

Device layout: hidden units on SBUF partitions (H = 128), agents on the free
dim.  All per-step matmuls are [128,128]^T @ [128, NT] with NT = 512 agents.
The agent_features transpose, all weight transposes, the control-input
contributions (constant over agents and time), and algebraic folds
(lecun-tanh scale into the ff/time weights, Wa+Wtb combine, forget-gate +1.0)
are done on the host in numpy before launching.
"""

import numpy as np

import concourse.bass as bass
import concourse.mybir as mybir
import concourse.tile as tile
from concourse.alu_op_type import AluOpType
from concourse.bass_utils import run_bass_kernel_spmd

D, H, BBU, T = 512, 128, 128, 10
N_FULL = 32768
N_CORES = 8
NT = 512  # agents per on-chip tile
CBLOB = 3344  # packed consts blob columns
F32 = mybir.dt.float32
F32R = mybir.dt.float32r
AF = mybir.ActivationFunctionType
OP = AluOpType


def build_bass(na, iters=1, reps=1, ablate=(), small_aft=False, timing_dummy=False):
    """Emit the per-core program for `na` agents (na % NT == 0).

    iters > 1 wraps the whole body in a device-side For loop that recomputes
    the identical result `iters` times -- used only to measure the per-body
    execution time from host wall-clock deltas (no NTFF profiling under this
    axon client)."""
    assert na % NT == 0
    ntiles = na // NT
    nsub = NT // 128  # 128-agent subtiles per tile (readout granularity)

    nc = bass.Bass("TRN2", debug=False, target_bir_lowering=False)

    # ---- DRAM I/O (all float32; host pre-transposes / pre-folds) ----
    # one packed constants blob -> a single DMA (column layout must match
    # pack_blob() in prep_inputs)
    afT_cols = NT if small_aft else na
    afT_d = nc.dram_tensor("afT", [D, afT_cols], F32R, kind="ExternalInput")
    blob_d = nc.dram_tensor("blob", [128, CBLOB], F32R, kind="ExternalInput")
    out_d = nc.dram_tensor("out", [na, 2 * T], F32, kind="ExternalOutput")
    dummy_d = (
        nc.dram_tensor("tdum", [1, 128], F32, kind="ExternalInput")
        if timing_dummy
        else None
    )

    from contextlib import nullcontext

    # timing-bisection ablations: stub out whole engine op classes
    if "mm" in ablate:
        nc.tensor.matmul = lambda *a, **k: None
    if "act" in ablate:
        nc.scalar.activation = lambda *a, **k: None
    if "dve" in ablate:
        nc.vector.tensor_tensor = lambda *a, **k: None
        nc.vector.scalar_tensor_tensor = lambda *a, **k: None
        nc.vector.tensor_scalar_add = lambda *a, **k: None
        nc.vector.tensor_copy = lambda *a, **k: None
    if "dma" in ablate:
        nc.sync.dma_start = lambda *a, **k: None

    with tile.TileContext(nc) as tc:
        with (
            tc.tile_pool(name="consts", bufs=1) as cp,
            tc.tile_pool(name="sb", bufs=3) as sb,
            tc.tile_pool(name="state", bufs=1) as st,
            tc.tile_pool(name="ps", bufs=1, space="PSUM") as ps,
            (tc.For_i(0, iters, 1) if iters > 1 else nullcontext()),
        ):
            # ---- prefetch tile 0's agent features ahead of the consts blob
            # (shortens the init-critical DMA chain at kernel start) ----
            afT0 = sb.tile([128, D // 128, NT], F32R, name="afT", tag="afT", bufs=4)
            nc.sync.dma_start(
                afT0[:], afT_d[:, 0:NT].rearrange("(k p) n -> p k n", p=128)
            )
            afT0_holder = [afT0]
            # ---- constants to SBUF in one DMA ----
            blob = cp.tile([128, CBLOB], F32R, name="blob")
            nc.sync.dma_start(blob[:], blob_d[:])
            if dummy_d is not None:
                dummy_t = cp.tile([1, 128], F32, name="dummy_t")
                nc.sync.dma_start(dummy_t[:], dummy_d[:])
            wrT = blob[:, 0:512]
            winitT = blob[:, 512:1536].rearrange("p (k m) -> p k m", k=4)
            wbhT = blob[:, 1536:1664]
            w1T = blob[:, 1664:1792]
            w2T = blob[:, 1792:1920]
            wabT = blob[:, 1920:2048]
            wfcT = blob[:, 2048:2050]
            bcols = blob[:, 2050:2055].bitcast(F32)
            brows = blob[0:1, 2056 : 2056 + 6 * H + 2 * nsub]
            ones = blob[0:1, 2832 : 2832 + NT]

            # ---- per-tile persistent state ----
            cs = [None] * ntiles  # LSTM cell state per tile
            ysb = [None] * ntiles  # readout accumulation per tile
            hs = [None] * ntiles  # current h AP per tile

            def alloc_state():
                for j in range(ntiles):
                    cs[j] = st.tile([H, NT], F32, name=f"c{j}", tag=f"c{j}", bufs=1)
                    ysb[j] = st.tile(
                        [128, nsub, 2 * T], F32, name=f"ysb{j}", tag=f"ysb{j}", bufs=1
                    )

            def h_tile(j):
                return sb.tile([H, NT], F32R, name=f"h{j}", tag=f"h{j}", bufs=2)

            def emit_init(j):
                # init psum lives in the bb/tgy banks (not the gate z banks)
                # so phase-A inits never serialize against the gate pipeline.
                if j == 0 and afT0_holder:
                    afT = afT0_holder.pop()  # prefetched before the consts blob
                else:
                    afT = sb.tile(
                        [128, D // 128, NT], F32R, name="afT", tag="afT", bufs=4
                    )
                    src_col = 0 if small_aft else j * NT
                    nc.sync.dma_start(
                        afT[:],
                        afT_d[:, src_col : src_col + NT].rearrange(
                            "(k p) n -> p k n", p=128
                        ),
                    )
                zh = ps.tile([128, NT], F32, name="zi_h", tag="bb")
                zc = ps.tile([128, NT], F32, name="zi_c", tag="tgy")
                for b, zp in enumerate((zh, zc)):
                    for k in range(D // 128):
                        nc.tensor.matmul(
                            zp[:],
                            (winitT[:, k, b * H : (b + 1) * H]),
                            (afT[:, k, :]),
                            start=(k == 0),
                            stop=(k == D // 128 - 1),
                        )
                h0 = h_tile(j)
                nc.vector.tensor_scalar_add(h0[:], zh[:], bcols[:, 3:4])
                nc.vector.tensor_scalar_add(cs[j][:], zc[:], bcols[:, 4:5])
                hs[j] = h0

            # ---- recurrence, step-major across tiles ----
            # readout matmuls/copies for a block are emitted inside the NEXT
            # block so the in-order engine streams never stall on them.
            pending = None  # (t, j) whose readout has not been emitted yet

            def emit_y_mms(t, j):
                yp = ps.tile([128, nsub, 2], F32, name="yps", tag="tgy")
                # bias: out[p, (s, c)] = bfc[c], via ones-column x bfc-row
                nc.tensor.matmul(
                    yp[:],
                    (ones[:, :128]),
                    (brows[:, 6 * H : 6 * H + 2 * nsub]),
                    start=True,
                    stop=False,
                )
                for s in range(nsub):
                    nc.tensor.matmul(
                        yp[:, s, :],
                        (hs[j][:, s * 128 : (s + 1) * 128]),
                        (wfcT[:]),
                        start=False,
                        stop=(s == nsub - 1),
                    )
                return yp

            def emit_y_copy(t, j, yp):
                # yps [128, nsub, 2] -> ysb[j][:, :, 2t:2t+2]
                nc.vector.tensor_copy(ysb[j][:, :, 2 * t : 2 * t + 2], yp[:])

            def emit_step(t, j):
                nonlocal pending
                hprev = hs[j]
                # 1. gates: z = Wr @ h + row-bias via K=1 matmuls.
                # tanh gate computed as 2*sigmoid(2x)-1 so all 4 gates share
                # one merged Sigmoid activation (Wr i-block and b_i pre-scaled
                # by 2 on the host).
                z = ps.tile([128, 4, NT], F32, name="z", tag="z")
                for g in range(4):
                    nc.tensor.matmul(
                        z[:, g, :],
                        (wrT[:, g * H : (g + 1) * H]),
                        (hprev[:]),
                        start=True,
                        stop=False,
                    )
                    nc.tensor.matmul(
                        z[:, g, :],
                        (brows[:, g * H : (g + 1) * H]),
                        (ones[:]),
                        start=False,
                        stop=True,
                    )
                # 2. merged gate activation
                sig4 = sb.tile([H, 4, NT], F32, name="sig4", tag="sig4")
                nc.scalar.activation(sig4[:], z[:], AF.Sigmoid)
                # 3. c update: c = (2*sig(2i)-1)*sig(ig) + c*sig(fg)
                m1 = sb.tile([H, NT], F32, name="m1", tag="m1")
                nc.vector.scalar_tensor_tensor(
                    m1[:], sig4[:, 0, :], 0.5, sig4[:, 1, :], OP.subtract, OP.mult
                )
                m2 = sb.tile([H, NT], F32, name="m2", tag="m2")
                nc.vector.tensor_tensor(m2[:], cs[j][:], sig4[:, 2, :], OP.mult)
                nc.vector.scalar_tensor_tensor(
                    cs[j][:], m1[:], 2.0, m2[:], OP.mult, OP.add
                )
                # 4. h_lstm
                tanh_c = sb.tile([H, NT], F32, name="tanh_c", tag="tanh_c")
                nc.scalar.activation(tanh_c[:], cs[j][:], AF.Tanh)
                hl = sb.tile([H, NT], F32R, name="hl", tag="hl")
                nc.vector.tensor_tensor(hl[:], tanh_c[:], sig4[:, 3, :], OP.mult)
                # 5. backbone
                bbp = ps.tile([128, NT], F32, name="bbp", tag="bb")
                nc.tensor.matmul(bbp[:], (wbhT[:]), (hl[:]), start=True, stop=True)
                # 5b. deferred readout of the previous block
                ypp = None
                if pending is not None:
                    ypp = emit_y_mms(*pending)
                x = sb.tile([BBU, NT], F32R, name="x", tag="x")
                nc.scalar.activation(x[:], bbp[:], AF.Tanh, bias=bcols[:, 1:2])
                if pending is not None:
                    emit_y_copy(*pending, ypp)
                    pending = None
                # 6. ff1 / ff2 (row-bias matmuls) and time-gate
                ffp = ps.tile([128, 2, NT], F32, name="ffp", tag="ff")
                for q, wT in enumerate((w1T, w2T)):
                    nc.tensor.matmul(
                        ffp[:, q, :], (wT[:]), (x[:]), start=True, stop=False
                    )
                    nc.tensor.matmul(
                        ffp[:, q, :],
                        (brows[:, (4 + q) * H : (5 + q) * H]),
                        (ones[:]),
                        start=False,
                        stop=True,
                    )
                tgp = ps.tile([128, NT], F32, name="tgp", tag="tgy")
                nc.tensor.matmul(tgp[:], (wabT[:]), (x[:]), start=True, stop=True)
                ff = sb.tile([H, 2, NT], F32, name="ff", tag="ff")
                nc.scalar.activation(ff[:], ffp[:], AF.Tanh)
                tg = sb.tile([H, NT], F32, name="tg", tag="tg")
                nc.scalar.activation(tg[:], tgp[:], AF.Sigmoid, bias=bcols[:, 2:3])
                # 7. h_new = ff1 + tg * (ff2 - ff1)
                dd = sb.tile([H, NT], F32, name="dd", tag="dd")
                nc.vector.tensor_tensor(dd[:], ff[:, 1, :], ff[:, 0, :], OP.subtract)
                ee = sb.tile([H, NT], F32, name="ee", tag="ee")
                nc.vector.tensor_tensor(ee[:], tg[:], dd[:], OP.mult)
                hnew = h_tile(j)
                nc.vector.tensor_tensor(hnew[:], ff[:, 0, :], ee[:], OP.add)
                hs[j] = hnew
                pending = (t, j)

            # drive: inits staggered two tiles ahead of their step 0 so the
            # gate/psum pipelines never wait on init work, then step-major.
            # reps > 1 re-emits the whole body (timing builds only).
            for rep in range(reps):
                alloc_state()
                for j in range(min(2, ntiles)):
                    emit_init(j)
                for j in range(ntiles):
                    emit_step(0, j)
                    if j + 2 < ntiles:
                        emit_init(j + 2)
                for t in range(1, T):
                    for j in range(ntiles):
                        emit_step(t, j)

                # final block's readout
                yp = emit_y_mms(*pending)
                emit_y_copy(*pending, yp)
                pending = None

                # ---- store ----
                for j in range(ntiles):
                    nc.sync.dma_start(
                        out_d[j * NT : (j + 1) * NT, :].rearrange(
                            "(s p) u -> p s u", p=128
                        ),
                        ysb[j][:],
                    )

    _split_excess_waits(nc)
    nc.finalize()
    return nc


def _split_excess_waits(nc):
    """This walrus build rejects instructions with more than one embedded sem
    wait ("Too many sync wait commands"); offload the excess onto single-wait
    drains inserted just before the offender on the same engine."""
    for fn in nc.m.functions:
        for bb in fn.blocks:
            insts = bb.instructions
            out = []
            changed = False
            for inst in insts:
                si = inst.sync_info
                maxw = 1  # this walrus build: one embedded wait per instruction

                if si is not None and len(si.on_wait) > maxw:
                    waits = list(si.on_wait)
                    excess, keep = waits[:-maxw], waits[-maxw:]
                    for k in range(len(excess)):
                        carrier = mybir.InstDrain(
                            name=nc.get_next_instruction_name(),
                            ins=[],
                            outs=[],
                            bass_is_fusable=False,
                        )
                        carrier.engine = inst.engine
                        carrier.sync_info = mybir.SyncInfo(
                            on_wait=[excess[k]], on_update=[]
                        )
                        nc.register_instruction(carrier)
                        out.append(carrier)
                    inst.sync_info = mybir.SyncInfo(
                        on_wait=keep, on_update=list(si.on_update)
                    )
                    changed = True
                out.append(inst)
            if changed:
                insts[:] = out


def prep_inputs(
    agent_features,
    control_input,
    W_init,
    b_init,
    Wi,
    bi,
    Wr,
    Wb,
    bb,
    W1,
    b1,
    W2,
    b2,
    Wa,
    ba,
    Wtb,
    btb,
    Wfc,
    bfc,
    na=None,
    n_cores=N_CORES,
):
    """Host-side preprocessing: transposes, folds, per-core shards."""
    f32 = np.float32
    n = agent_features.shape[0]
    na = na or n // n_cores
    nsub = NT // 128

    lstm_b = (control_input.astype(np.float64) @ Wi.T.astype(np.float64)) + bi
    lstm_b = lstm_b.astype(f32).copy()
    lstm_b[2 * H : 3 * H] += 1.0  # ncps forget-gate bias
    xb = 0.666 * ((control_input.astype(np.float64) @ Wb[:, :D].T.astype(np.float64)) + bb)
    xb = xb.astype(f32)

    wbh = (0.666 * Wb[:, D:]).astype(f32)  # [BB, H]
    w1s = (1.7159 * W1).astype(f32)
    w2s = (1.7159 * W2).astype(f32)
    wabs = (1.7159 * (Wa + Wtb)).astype(f32)
    bab = (ba + btb).astype(f32)

    bcols = np.stack(
        [lstm_b[0:H], xb, bab, b_init[:H].astype(f32), b_init[H:].astype(f32)], axis=1
    ).astype(f32)  # [128, 5]
    brows = np.concatenate(
        [2.0 * lstm_b[0:H], lstm_b[H : 2 * H], lstm_b[2 * H : 3 * H],
         lstm_b[3 * H : 4 * H], b1.astype(f32), b2.astype(f32),
         np.tile(bfc.astype(f32), nsub)]
    ).astype(f32)[None, :]  # [1, 6H + 2*nsub]

    wrT = Wr.T.astype(f32) * np.repeat([2.0, 1.0, 1.0, 1.0], H)[None, :].astype(f32)
    winitT = W_init.T.astype(f32)  # [D, 2H]
    # pack the consts blob; layout mirrored in build_bass
    blob = np.zeros((128, CBLOB), f32)
    blob[:, 0:512] = wrT
    blob[:, 512:1536] = winitT.reshape(4, 128, 2 * H).transpose(1, 0, 2).reshape(128, 1024)
    blob[:, 1536:1664] = wbh.T
    blob[:, 1664:1792] = w1s.T
    blob[:, 1792:1920] = w2s.T
    blob[:, 1920:2048] = wabs.T
    blob[:, 2048:2050] = Wfc.T.astype(f32)
    blob[:, 2050:2055] = bcols
    blob[0, 2056 : 2056 + 6 * H + 2 * nsub] = brows[0]
    blob[0, 2832 : 2832 + NT] = 1.0
    in_maps = []
    n_shards = n // na
    for c in range(n_shards):
        af_c = agent_features[c * na : (c + 1) * na].astype(f32)
        m = {"blob": blob, "afT": np.ascontiguousarray(af_c.T)}
        in_maps.append(m)
    return in_maps


def run(inputs, trace=False, **kwargs):
    """Compile + run on all 8 cores.  Returns (output [N, T, 2], results)."""
    n = inputs["agent_features"].shape[0]
    na = n // N_CORES
    nc = build_bass(na)
    in_maps = prep_inputs(**inputs, na=na)
    res = run_bass_kernel_spmd(
        nc, in_maps, core_ids=list(range(N_CORES)), trace=trace, **kwargs
    )
    out = np.concatenate([r["out"] for r in res.results], axis=0)
    return out.reshape(n, T, 2), res


def kernel(**inputs):
    inputs = {k: np.asarray(v) for k, v in inputs.items()}
    out, _ = run(inputs)
    return out.astype(np.float32)


# revision 24
# speedup vs baseline: 1.0189x; 1.0189x over previous
"""Trainium2 Bass kernel for nn_AgentPredictionHead.

Model: per-agent CfC (closed-form continuous-time) prediction head:
  - initial (h, c) from agent_features via Linear(D, 2H)
  - T steps of a mixed-memory LSTM cell + CfC cell (control input constant)
  - per-step readout Linear(H, 2)

Sharding: data-parallel over agents across 8 NeuronCores (weights replicated).

Device layout: hidden units on SBUF partitions (H = 128), agents on the free
dim.  All per-step matmuls are [128,128]^T @ [128, NT] with NT = 512 agents.
The agent_features transpose, all weight transposes, the control-input
contributions (constant over agents and time), and algebraic folds
(lecun-tanh scale into the ff/time weights, Wa+Wtb combine, forget-gate +1.0)
are done on the host in numpy before launching.
"""

import numpy as np

import concourse.bass as bass
import concourse.mybir as mybir
import concourse.tile as tile
from concourse.alu_op_type import AluOpType
from concourse.bass_utils import run_bass_kernel_spmd

D, H, BBU, T = 512, 128, 128, 10
N_FULL = 32768
N_CORES = 8
NT = 512  # agents per on-chip tile
CBLOB = 3344  # packed consts blob columns
F32 = mybir.dt.float32
F32R = mybir.dt.float32r
AF = mybir.ActivationFunctionType
OP = AluOpType


def build_bass(na, iters=1, reps=1, ablate=(), small_aft=False, timing_dummy=False):
    """Emit the per-core program for `na` agents (na % NT == 0).

    iters > 1 wraps the whole body in a device-side For loop that recomputes
    the identical result `iters` times -- used only to measure the per-body
    execution time from host wall-clock deltas (no NTFF profiling under this
    axon client)."""
    assert na % NT == 0
    ntiles = na // NT
    nsub = NT // 128  # 128-agent subtiles per tile (readout granularity)

    nc = bass.Bass("TRN2", debug=False, target_bir_lowering=False)

    # ---- DRAM I/O (all float32; host pre-transposes / pre-folds) ----
    # one packed constants blob -> a single DMA (column layout must match
    # pack_blob() in prep_inputs)
    afT_cols = NT if small_aft else na
    afT_d = nc.dram_tensor("afT", [D, afT_cols], F32R, kind="ExternalInput")
    blob_d = nc.dram_tensor("blob", [128, CBLOB], F32R, kind="ExternalInput")
    out_d = nc.dram_tensor("out", [na, 2 * T], F32, kind="ExternalOutput")
    dummy_d = (
        nc.dram_tensor("tdum", [1, 128], F32, kind="ExternalInput")
        if timing_dummy
        else None
    )

    from contextlib import nullcontext

    # timing-bisection ablations: stub out whole engine op classes
    if "mm" in ablate:
        nc.tensor.matmul = lambda *a, **k: None
    if "act" in ablate:
        nc.scalar.activation = lambda *a, **k: None
    if "dve" in ablate:
        nc.vector.tensor_tensor = lambda *a, **k: None
        nc.vector.scalar_tensor_tensor = lambda *a, **k: None
        nc.vector.tensor_scalar_add = lambda *a, **k: None
        nc.vector.tensor_copy = lambda *a, **k: None
    if "dma" in ablate:
        nc.sync.dma_start = lambda *a, **k: None

    with tile.TileContext(nc) as tc:
        with (
            tc.tile_pool(name="consts", bufs=1) as cp,
            tc.tile_pool(name="sb", bufs=4) as sb,
            tc.tile_pool(name="state", bufs=1) as st,
            tc.tile_pool(name="ps", bufs=1, space="PSUM") as ps,
            (tc.For_i(0, iters, 1) if iters > 1 else nullcontext()),
        ):
            # ---- prefetch tile 0's agent features ahead of the consts blob
            # (shortens the init-critical DMA chain at kernel start) ----
            afT0 = sb.tile([128, D // 128, NT], F32R, name="afT", tag="afT", bufs=4)
            nc.sync.dma_start(
                afT0[:], afT_d[:, 0:NT].rearrange("(k p) n -> p k n", p=128)
            )
            afT0_holder = [afT0]
            # ---- constants to SBUF in one DMA ----
            blob = cp.tile([128, CBLOB], F32R, name="blob")
            nc.sync.dma_start(blob[:], blob_d[:])
            if dummy_d is not None:
                dummy_t = cp.tile([1, 128], F32, name="dummy_t")
                nc.sync.dma_start(dummy_t[:], dummy_d[:])
            wrT = blob[:, 0:512]
            winitT = blob[:, 512:1536].rearrange("p (k m) -> p k m", k=4)
            wbhT = blob[:, 1536:1664]
            w1T = blob[:, 1664:1792]
            w2T = blob[:, 1792:1920]
            wabT = blob[:, 1920:2048]
            wfcT = blob[:, 2048:2050]
            bcols = blob[:, 2050:2055].bitcast(F32)
            brows = blob[0:1, 2056 : 2056 + 6 * H + 2 * nsub]
            ones = blob[0:1, 2832 : 2832 + NT]

            # ---- per-tile persistent state ----
            cs = [None] * ntiles  # LSTM cell state per tile
            ysb = [None] * ntiles  # readout accumulation per tile
            hs = [None] * ntiles  # current h AP per tile

            def alloc_state():
                for j in range(ntiles):
                    cs[j] = st.tile([H, NT], F32, name=f"c{j}", tag=f"c{j}", bufs=1)
                    ysb[j] = st.tile(
                        [128, nsub, 2 * T], F32, name=f"ysb{j}", tag=f"ysb{j}", bufs=1
                    )

            def h_tile(j):
                return sb.tile([H, NT], F32R, name=f"h{j}", tag=f"h{j}", bufs=2)

            def emit_init(j):
                # init psum lives in the bb/tgy banks (not the gate z banks)
                # so phase-A inits never serialize against the gate pipeline.
                if j == 0 and afT0_holder:
                    afT = afT0_holder.pop()  # prefetched before the consts blob
                else:
                    afT = sb.tile(
                        [128, D // 128, NT], F32R, name="afT", tag="afT", bufs=4
                    )
                    src_col = 0 if small_aft else j * NT
                    nc.sync.dma_start(
                        afT[:],
                        afT_d[:, src_col : src_col + NT].rearrange(
                            "(k p) n -> p k n", p=128
                        ),
                    )
                zh = ps.tile([128, NT], F32, name="zi_h", tag="bb")
                zc = ps.tile([128, NT], F32, name="zi_c", tag="tgy")
                for b, zp in enumerate((zh, zc)):
                    for k in range(D // 128):
                        nc.tensor.matmul(
                            zp[:],
                            (winitT[:, k, b * H : (b + 1) * H]),
                            (afT[:, k, :]),
                            start=(k == 0),
                            stop=(k == D // 128 - 1),
                        )
                h0 = h_tile(j)
                nc.vector.tensor_scalar_add(h0[:], zh[:], bcols[:, 3:4])
                nc.vector.tensor_scalar_add(cs[j][:], zc[:], bcols[:, 4:5])
                hs[j] = h0

            # ---- recurrence, step-major across tiles ----
            # readout matmuls/copies for a block are emitted inside the NEXT
            # block so the in-order engine streams never stall on them.
            pending = None  # (t, j) whose readout has not been emitted yet

            def emit_y_mms(t, j):
                yp = ps.tile([128, nsub, 2], F32, name="yps", tag="tgy")
                # bias: out[p, (s, c)] = bfc[c], via ones-column x bfc-row
                nc.tensor.matmul(
                    yp[:],
                    (ones[:, :128]),
                    (brows[:, 6 * H : 6 * H + 2 * nsub]),
                    start=True,
                    stop=False,
                )
                for s in range(nsub):
                    nc.tensor.matmul(
                        yp[:, s, :],
                        (hs[j][:, s * 128 : (s + 1) * 128]),
                        (wfcT[:]),
                        start=False,
                        stop=(s == nsub - 1),
                    )
                return yp

            def emit_y_copy(t, j, yp):
                # yps [128, nsub, 2] -> ysb[j][:, :, 2t:2t+2]
                nc.vector.tensor_copy(ysb[j][:, :, 2 * t : 2 * t + 2], yp[:])

            def emit_step(t, j):
                nonlocal pending
                hprev = hs[j]
                # 1. gates: z = Wr @ h + row-bias via K=1 matmuls.
                # tanh gate computed as 2*sigmoid(2x)-1 so all 4 gates share
                # one merged Sigmoid activation (Wr i-block and b_i pre-scaled
                # by 2 on the host).
                z = ps.tile([128, 4, NT], F32, name="z", tag="z")
                for g in range(4):
                    nc.tensor.matmul(
                        z[:, g, :],
                        (wrT[:, g * H : (g + 1) * H]),
                        (hprev[:]),
                        start=True,
                        stop=False,
                    )
                    nc.tensor.matmul(
                        z[:, g, :],
                        (brows[:, g * H : (g + 1) * H]),
                        (ones[:]),
                        start=False,
                        stop=True,
                    )
                # 2. merged gate activation
                sig4 = sb.tile([H, 4, NT], F32, name="sig4", tag="sig4", bufs=3)
                nc.scalar.activation(sig4[:], z[:], AF.Sigmoid)
                # 3. c update: c = (2*sig(2i)-1)*sig(ig) + c*sig(fg)
                m1 = sb.tile([H, NT], F32, name="m1", tag="m1")
                nc.vector.scalar_tensor_tensor(
                    m1[:], sig4[:, 0, :], 0.5, sig4[:, 1, :], OP.subtract, OP.mult
                )
                m2 = sb.tile([H, NT], F32, name="m2", tag="m2")
                nc.vector.tensor_tensor(m2[:], cs[j][:], sig4[:, 2, :], OP.mult)
                nc.vector.scalar_tensor_tensor(
                    cs[j][:], m1[:], 2.0, m2[:], OP.mult, OP.add
                )
                # 4. h_lstm
                tanh_c = sb.tile([H, NT], F32, name="tanh_c", tag="tanh_c")
                nc.scalar.activation(tanh_c[:], cs[j][:], AF.Tanh)
                hl = sb.tile([H, NT], F32R, name="hl", tag="hl")
                nc.vector.tensor_tensor(hl[:], tanh_c[:], sig4[:, 3, :], OP.mult)
                # 5. backbone
                bbp = ps.tile([128, NT], F32, name="bbp", tag="bb")
                nc.tensor.matmul(bbp[:], (wbhT[:]), (hl[:]), start=True, stop=True)
                # 5b. deferred readout of the previous block
                ypp = None
                if pending is not None:
                    ypp = emit_y_mms(*pending)
                x = sb.tile([BBU, NT], F32R, name="x", tag="x")
                nc.scalar.activation(x[:], bbp[:], AF.Tanh, bias=bcols[:, 1:2])
                if pending is not None:
                    emit_y_copy(*pending, ypp)
                    pending = None
                # 6. ff1 / ff2 (row-bias matmuls) and time-gate
                ffp = ps.tile([128, 2, NT], F32, name="ffp", tag="ff")
                for q, wT in enumerate((w1T, w2T)):
                    nc.tensor.matmul(
                        ffp[:, q, :], (wT[:]), (x[:]), start=True, stop=False
                    )
                    nc.tensor.matmul(
                        ffp[:, q, :],
                        (brows[:, (4 + q) * H : (5 + q) * H]),
                        (ones[:]),
                        start=False,
                        stop=True,
                    )
                tgp = ps.tile([128, NT], F32, name="tgp", tag="tgy")
                nc.tensor.matmul(tgp[:], (wabT[:]), (x[:]), start=True, stop=True)
                ff = sb.tile([H, 2, NT], F32, name="ff", tag="ff", bufs=3)
                nc.scalar.activation(ff[:], ffp[:], AF.Tanh)
                tg = sb.tile([H, NT], F32, name="tg", tag="tg")
                nc.scalar.activation(tg[:], tgp[:], AF.Sigmoid, bias=bcols[:, 2:3])
                # 7. h_new = ff1 + tg * (ff2 - ff1)
                dd = sb.tile([H, NT], F32, name="dd", tag="dd")
                nc.vector.tensor_tensor(dd[:], ff[:, 1, :], ff[:, 0, :], OP.subtract)
                ee = sb.tile([H, NT], F32, name="ee", tag="ee")
                nc.vector.tensor_tensor(ee[:], tg[:], dd[:], OP.mult)
                hnew = h_tile(j)
                nc.vector.tensor_tensor(hnew[:], ff[:, 0, :], ee[:], OP.add)
                hs[j] = hnew
                pending = (t, j)

            # drive: inits staggered two tiles ahead of their step 0 so the
            # gate/psum pipelines never wait on init work, then step-major.
            # reps > 1 re-emits the whole body (timing builds only).
            for rep in range(reps):
                alloc_state()
                for j in range(min(2, ntiles)):
                    emit_init(j)
                for j in range(ntiles):
                    emit_step(0, j)
                    if j + 2 < ntiles:
                        emit_init(j + 2)
                for t in range(1, T):
                    for j in range(ntiles):
                        emit_step(t, j)

                # final block's readout
                yp = emit_y_mms(*pending)
                emit_y_copy(*pending, yp)
                pending = None

                # ---- store ----
                for j in range(ntiles):
                    nc.sync.dma_start(
                        out_d[j * NT : (j + 1) * NT, :].rearrange(
                            "(s p) u -> p s u", p=128
                        ),
                        ysb[j][:],
                    )

    _split_excess_waits(nc)
    nc.finalize()
    return nc


def _split_excess_waits(nc):
    """This walrus build rejects instructions with more than one embedded sem
    wait ("Too many sync wait commands"); offload the excess onto single-wait
    drains inserted just before the offender on the same engine."""
    for fn in nc.m.functions:
        for bb in fn.blocks:
            insts = bb.instructions
            out = []
            changed = False
            for inst in insts:
                si = inst.sync_info
                maxw = 1  # this walrus build: one embedded wait per instruction

                if si is not None and len(si.on_wait) > maxw:
                    waits = list(si.on_wait)
                    excess, keep = waits[:-maxw], waits[-maxw:]
                    for k in range(len(excess)):
                        carrier = mybir.InstDrain(
                            name=nc.get_next_instruction_name(),
                            ins=[],
                            outs=[],
                            bass_is_fusable=False,
                        )
                        carrier.engine = inst.engine
                        carrier.sync_info = mybir.SyncInfo(
                            on_wait=[excess[k]], on_update=[]
                        )
                        nc.register_instruction(carrier)
                        out.append(carrier)
                    inst.sync_info = mybir.SyncInfo(
                        on_wait=keep, on_update=list(si.on_update)
                    )
                    changed = True
                out.append(inst)
            if changed:
                insts[:] = out


def prep_inputs(
    agent_features,
    control_input,
    W_init,
    b_init,
    Wi,
    bi,
    Wr,
    Wb,
    bb,
    W1,
    b1,
    W2,
    b2,
    Wa,
    ba,
    Wtb,
    btb,
    Wfc,
    bfc,
    na=None,
    n_cores=N_CORES,
):
    """Host-side preprocessing: transposes, folds, per-core shards."""
    f32 = np.float32
    n = agent_features.shape[0]
    na = na or n // n_cores
    nsub = NT // 128

    lstm_b = (control_input.astype(np.float64) @ Wi.T.astype(np.float64)) + bi
    lstm_b = lstm_b.astype(f32).copy()
    lstm_b[2 * H : 3 * H] += 1.0  # ncps forget-gate bias
    xb = 0.666 * ((control_input.astype(np.float64) @ Wb[:, :D].T.astype(np.float64)) + bb)
    xb = xb.astype(f32)

    wbh = (0.666 * Wb[:, D:]).astype(f32)  # [BB, H]
    w1s = (1.7159 * W1).astype(f32)
    w2s = (1.7159 * W2).astype(f32)
    wabs = (1.7159 * (Wa + Wtb)).astype(f32)
    bab = (ba + btb).astype(f32)

    bcols = np.stack(
        [lstm_b[0:H], xb, bab, b_init[:H].astype(f32), b_init[H:].astype(f32)], axis=1
    ).astype(f32)  # [128, 5]
    brows = np.concatenate(
        [2.0 * lstm_b[0:H], lstm_b[H : 2 * H], lstm_b[2 * H : 3 * H],
         lstm_b[3 * H : 4 * H], b1.astype(f32), b2.astype(f32),
         np.tile(bfc.astype(f32), nsub)]
    ).astype(f32)[None, :]  # [1, 6H + 2*nsub]

    wrT = Wr.T.astype(f32) * np.repeat([2.0, 1.0, 1.0, 1.0], H)[None, :].astype(f32)
    winitT = W_init.T.astype(f32)  # [D, 2H]
    # pack the consts blob; layout mirrored in build_bass
    blob = np.zeros((128, CBLOB), f32)
    blob[:, 0:512] = wrT
    blob[:, 512:1536] = winitT.reshape(4, 128, 2 * H).transpose(1, 0, 2).reshape(128, 1024)
    blob[:, 1536:1664] = wbh.T
    blob[:, 1664:1792] = w1s.T
    blob[:, 1792:1920] = w2s.T
    blob[:, 1920:2048] = wabs.T
    blob[:, 2048:2050] = Wfc.T.astype(f32)
    blob[:, 2050:2055] = bcols
    blob[0, 2056 : 2056 + 6 * H + 2 * nsub] = brows[0]
    blob[0, 2832 : 2832 + NT] = 1.0
    in_maps = []
    n_shards = n // na
    for c in range(n_shards):
        af_c = agent_features[c * na : (c + 1) * na].astype(f32)
        m = {"blob": blob, "afT": np.ascontiguousarray(af_c.T)}
        in_maps.append(m)
    return in_maps


def run(inputs, trace=False, **kwargs):
    """Compile + run on all 8 cores.  Returns (output [N, T, 2], results)."""
    n = inputs["agent_features"].shape[0]
    na = n // N_CORES
    nc = build_bass(na)
    in_maps = prep_inputs(**inputs, na=na)
    res = run_bass_kernel_spmd(
        nc, in_maps, core_ids=list(range(N_CORES)), trace=trace, **kwargs
    )
    out = np.concatenate([r["out"] for r in res.results], axis=0)
    return out.reshape(n, T, 2), res


def kernel(**inputs):
    inputs = {k: np.asarray(v) for k, v in inputs.items()}
    out, _ = run(inputs)
    return out.astype(np.float32)
